# revision 13
# baseline (speedup 1.0000x reference)
"""Trainium2 kernel for nn_CustomConv1d_6150393168147.

Key algebraic simplification: in the reference, ``on_diag[i, o] =
((o + i) % 256 == o)`` is nonzero only for i == 0 (C_IN == C_OUT == 256),
so

    diag_vals[o] = alpha_topk[0] * V[0, o]
    W[o, c, k]   = diag_vals[o] * (c == o)      for all k in {0, 1, 2}

and the "conv" collapses to a per-channel 3-tap box filter:

    out[n, c, t] = scale[c] * (x[n,c,t-1] + x[n,c,t] + x[n,c,t+1]) + bias[c]

with zero padding at the ends, scale[c] = alpha_topk[0] * V[0, c].

The Dykstra top-k projection only couples channels through a scalar sum,
is O(C * n_iter) and is computed on the host (float32, faithful to the
reference op-for-op).  The heavy streaming part (8 x 256 x 16384 f32 in
and out = 256 MiB of HBM traffic) runs on 8 NeuronCores, data-parallel
over batch (1 batch element per core), and is HBM-bandwidth bound.

Per core: channels map to 2 partition blocks of 128; length is tiled
along the free dim with a 1-element halo.  Per tile:
  DVE :  s2 = x[t-1] + x[t+1]
  DVE/POOL (alternating): s3 = s2 + x[t]
  ACT :  y = Identity(s3 * scale + bias)   (per-partition scale/bias APs)
"""

import os
import sys

import numpy as np

for _p in ("/opt/trn_rl_repo", "/root/.axon_site/_ro/trn_rl_repo"):
    if os.path.isdir(_p) and _p not in sys.path:
        sys.path.insert(0, _p)

import concourse.bacc as bacc
import concourse.bass as bass
import concourse.mybir as mybir
from concourse.bass_utils import run_bass_kernel_spmd
from concourse.tile import TileContext

# Problem constants (hardcoded per the harness contract).
B, C, L = 8, 256, 16384
NCORES = 8
PBLK = C // 128  # partition blocks per core
K_TOP, ALPHA_LR, N_ITER = 16, 0.01, 50

TFREE = 4096  # free-dim tile size


def _alpha_topk0(alpha: np.ndarray) -> np.float32:
    """Dykstra sparse-soft-topk projection (float32, mirrors reference);
    returns element 0 of the projected vector, the only one used."""
    f32 = np.float32
    y = alpha.astype(np.float32) / f32(ALPHA_LR)
    p = np.zeros_like(y)
    q = np.zeros_like(y)
    n = f32(y.shape[0])
    k = f32(K_TOP)
    for _ in range(N_ITER):
        u = y + p
        z = u - (np.sum(u, dtype=np.float32) - k) / n
        p = u - z
        v = z + q
        y = np.clip(v, f32(0.0), f32(1.0))
        q = v - y
    return y[0]


_NC_CACHE = {}


def _build(repeats=1, tfree=TFREE, xbufs=4, ybufs=4, add2="vector", fin="act",
           tail_split=1, preload=False):
    """add2: engine for the second tensor add ("vector" | "gpsimd" | "alt").
    fin: engine/op for the scale+bias finalize ("act" | "vector" | "gpsimd").
    tail_split: split the program-final tile into this many sub-tiles so the
    end-of-kernel serial chain (load->add->add->fin->store) pipelines.
    preload: emit every load DMA before any compute/store.  Each load lands
    on its own HWDGE queue and dispatches immediately (needs xbufs >= number
    of tiles), so the input stream wins early bandwidth and the last store
    no longer waits on a bandwidth-starved last load."""
    key = (repeats, tfree, xbufs, ybufs, add2, fin, tail_split, preload)
    if key in _NC_CACHE:
        return _NC_CACHE[key]

    f32 = mybir.dt.float32
    # Bacc (not plain Bass): its finalize() runs generate_event_semaphores(),
    # which legalizes the TRN2 1-sync-wait-per-instruction cap.
    nc = bacc.Bacc(None, target_bir_lowering=False, debug=False, num_devices=NCORES)
    xd = nc.declare_dram_parameter("x", [PBLK, 128, L], f32, isOutput=False)
    sd = nc.declare_dram_parameter("scale", [PBLK, 128, 1], f32, isOutput=False)
    bd = nc.declare_dram_parameter("bias", [PBLK, 128, 1], f32, isOutput=False)
    od = nc.declare_dram_parameter("out", [PBLK, 128, L], f32, isOutput=True)

    nt = L // tfree
    with TileContext(nc) as tc:
        with (
            tc.tile_pool(name="const", bufs=1) as cpool,
            tc.tile_pool(name="xin", bufs=xbufs) as xpool,
            tc.tile_pool(name="yout", bufs=ybufs) as ypool,
        ):
            consts = []
            for b in range(PBLK):
                sct = cpool.tile([128, 1], f32, tag=f"sc{b}")
                bit = cpool.tile([128, 1], f32, tag=f"bi{b}")
                nc.sync.dma_start(out=sct[:], in_=sd[b])
                nc.sync.dma_start(out=bit[:], in_=bd[b])
                consts.append((sct, bit))

            base_segs = [(j * tfree, tfree) for j in range(nt)]
            tw = tfree // tail_split
            tail_segs = base_segs[:-1] + [
                (base_segs[-1][0] + i * tw, tw) for i in range(tail_split)
            ]

            def emit_load(b, t0, w):
                xt = xpool.tile([128, w + 2], f32, tag="x")
                if t0 == 0:
                    nc.vector.memset(xt[:, 0:1], 0.0)
                    nc.sync.dma_start(out=xt[:, 1 : w + 2], in_=xd[b, :, 0 : w + 1])
                elif t0 + w == L:
                    nc.vector.memset(xt[:, w + 1 : w + 2], 0.0)
                    nc.sync.dma_start(out=xt[:, 0 : w + 1], in_=xd[b, :, t0 - 1 : L])
                else:
                    nc.sync.dma_start(out=xt[:], in_=xd[b, :, t0 - 1 : t0 + w + 1])
                return xt

            def emit_compute(b, si, t0, w, xt):
                sct, bit = consts[b]
                yt = ypool.tile([128, w], f32, tag="y")
                nc.vector.tensor_add(
                    out=yt[:], in0=xt[:, 0:w], in1=xt[:, 2 : w + 2]
                )
                eng2 = {
                    "vector": nc.vector,
                    "gpsimd": nc.gpsimd,
                    "alt": nc.gpsimd if (si % 2) else nc.vector,
                }[add2]
                eng2.tensor_add(out=yt[:], in0=yt[:], in1=xt[:, 1 : w + 1])
                if fin == "act":
                    nc.scalar.activation(
                        out=yt[:],
                        in_=yt[:],
                        func=mybir.ActivationFunctionType.Identity,
                        bias=bit[:, 0:1],
                        scale=sct[:, 0:1],
                    )
                else:
                    feng = nc.vector if fin == "vector" else nc.gpsimd
                    feng.tensor_scalar(
                        out=yt[:],
                        in0=yt[:],
                        scalar1=sct[:, 0:1],
                        scalar2=bit[:, 0:1],
                        op0=mybir.AluOpType.mult,
                        op1=mybir.AluOpType.add,
                    )
                nc.sync.dma_start(out=od[b, :, t0 : t0 + w], in_=yt[:])

            for _rep in range(repeats):
                work = []
                for b in range(PBLK):
                    segs = tail_segs if b == PBLK - 1 else base_segs
                    for si, (t0, w) in enumerate(segs):
                        xt = emit_load(b, t0, w)
                        if preload:
                            work.append((b, si, t0, w, xt))
                        else:
                            emit_compute(b, si, t0, w, xt)
                for b, si, t0, w, xt in work:
                    emit_compute(b, si, t0, w, xt)

    nc.finalize()
    _NC_CACHE[key] = nc
    return nc


def run(x, V, alpha, bias, **spmd_kwargs):
    """Returns (out [B,C,L] f32, BassKernelResults)."""
    x = np.ascontiguousarray(np.asarray(x, dtype=np.float32))
    V = np.asarray(V, dtype=np.float32)
    alpha = np.asarray(alpha, dtype=np.float32)
    bias = np.asarray(bias, dtype=np.float32)

    a0 = _alpha_topk0(alpha)
    scale = (a0 * V[0, :]).astype(np.float32)  # [C]

    nc = _build()
    xs = x.reshape(B, PBLK, 128, L)
    sd = np.ascontiguousarray(scale.reshape(PBLK, 128, 1))
    bd = np.ascontiguousarray(bias.reshape(PBLK, 128, 1))
    in_maps = [{"x": xs[i], "scale": sd, "bias": bd} for i in range(NCORES)]
    res = run_bass_kernel_spmd(nc, in_maps, core_ids=list(range(NCORES)), **spmd_kwargs)
    out = np.stack(
        [np.asarray(res.results[i]["out"]).reshape(C, L) for i in range(NCORES)], axis=0
    )
    return out, res


def kernel(x, V, alpha, bias):
    out, _ = run(x, V, alpha, bias)
    return out
